# revision 26
# baseline (speedup 1.0000x reference)
"""BiMultiHeadAttention (vision-language cross attention) on 8 Trainium2 cores.

Strategy: pure data-parallel over batch (32 batches -> 4 per core).
All matmul data in fp16 (11-bit mantissa; errors ~1e-3 on the logit path),
accumulation in fp32 PSUM.  Layout chain keeps the contraction axis on
partitions at every step:

  vT/lT (host-pretransposed, fp16)
    -> qT,kT in [e, t] layout;  vv,lv in natural [t, e] layout
    -> S_T[s,t] and S2[t,s] both by matmul (bidirectional softmax needs both)
    -> E = exp(S - 4) in fp16 (shift keeps exp in fp16 range; softmax-invariant)
    -> PV matmuls produce un-normalized xv^T[e,t], xl^T[e,s] per head pair
    -> per-pair sumexp rows via one-hot matmuls; reciprocal; K=2 broadcast
       matmul builds the per-(head,column) scale tile; DVE multiply normalizes
    -> output projections back to natural layout, bias added as K=1 matmul rows.

The attention mask enters E_v as a per-partition bias (exact for any mask);
attn_l/softmax-over-queries is mask-free per the reference.
"""
import sys
import numpy as np

sys.path.insert(0, "/opt/trn_rl_repo")

B, VN, LN = 32, 576, 128
E, H, D = 1024, 16, 64
VD, LD = 1024, 768
NCORES = 8
BPC = B // NCORES          # batches per core
SCALE = H ** -0.5          # 0.25 (reference scales by num_heads**-0.5)
SHIFT = 4.0                # exp(logit - SHIFT) keeps E in fp16 range
TC = [(0, 128), (128, 128), (256, 128), (384, 128), (512, 64)]   # t chunks of 576
N576 = [(0, 512), (512, 64)]
N1024 = [(0, 512), (512, 512)]
N768 = [(0, 512), (512, 256)]

_cached_nc = None


def _oh16():
    m = np.zeros((128, 256), np.float16)
    for h in range(16):
        m[:, h * 16 + h] = 1.0
    return m


def _ind16():
    m = np.zeros((8, 16, 128), np.float16)
    for j in range(8):
        m[j, 2 * j, 0:64] = 1.0
        m[j, 2 * j + 1, 64:128] = 1.0
    return m
import os as _os
DBG_BPC = int(_os.environ.get("DBG_BPC", "0")) or None   # limit batches built
DBG_PAIRS = int(_os.environ.get("DBG_PAIRS", "0")) or None


def _build_nc():
    import concourse.bacc as bacc
    import concourse.hw_specs as hw_specs
    # Prefer the combined ln+exp ACT table set so alternating Ln/Exp
    # activations don't thrash ACT_TABLE_LOAD (~1.3us per reload).
    if not getattr(hw_specs, "_combined_ln_exp_first", False):
        _orig_get_tables = hw_specs.get_activation_tables

        def _reordered(arch):
            t = dict(_orig_get_tables(arch))   # order == act_info.json index order
            key = "natural_log_exp_and_others"
            if key in t:
                shared = t[key]
                strip = {f for f in shared}
                out = {}
                for k, v in t.items():
                    if k == key:
                        out[k] = v
                    else:
                        out[k] = v - strip if isinstance(v, set) else v
                return out
            return t

        hw_specs.get_activation_tables = _reordered
        bacc.get_activation_tables = _reordered
        hw_specs._combined_ln_exp_first = True
    import concourse.mybir as mybir
    import concourse.tile as tile
    from concourse.tile import add_dep_helper

    F16 = mybir.dt.float16
    F32 = mybir.dt.float32
    EXP = mybir.ActivationFunctionType.Exp
    LOG = mybir.ActivationFunctionType.Ln

    nc = bacc.Bacc("TRN2", target_bir_lowering=False, debug=False)
    vT = nc.declare_dram_parameter("vT", [BPC, VD, VN], F16, isOutput=False)
    lT = nc.declare_dram_parameter("lT", [LD, BPC * LN], F16, isOutput=False)
    wvq = nc.declare_dram_parameter("wvq", [VD, E], F16, isOutput=False)
    wlk = nc.declare_dram_parameter("wlk", [LD, E], F16, isOutput=False)
    wvv = nc.declare_dram_parameter("wvv", [VD, E], F16, isOutput=False)
    wlv = nc.declare_dram_parameter("wlv", [LD, E], F16, isOutput=False)
    wvo = nc.declare_dram_parameter("wvo", [E, VD], F16, isOutput=False)
    wlo = nc.declare_dram_parameter("wlo", [E, LD], F16, isOutput=False)
    bvq = nc.declare_dram_parameter("bvq", [1, E], F16, isOutput=False)
    blk = nc.declare_dram_parameter("blk", [1, E], F16, isOutput=False)
    bvv = nc.declare_dram_parameter("bvv", [1, E], F16, isOutput=False)
    blv = nc.declare_dram_parameter("blv", [1, E], F16, isOutput=False)
    bvo = nc.declare_dram_parameter("bvo", [1, VD], F16, isOutput=False)
    blo = nc.declare_dram_parameter("blo", [1, LD], F16, isOutput=False)
    maskb = nc.declare_dram_parameter("maskb", [LN, 1], F32, isOutput=False)
    ind2_d = nc.declare_dram_parameter("ind2", [2, 128], F16, isOutput=False)
    oh_d = nc.declare_dram_parameter("oh", [128, 2], F16, isOutput=False)
    oh2_d = nc.declare_dram_parameter("oh2", [128, 2], F16, isOutput=False)
    xv = nc.declare_dram_parameter("xv", [BPC, VN, VD], F32, isOutput=True)
    xl = nc.declare_dram_parameter("xl", [BPC, LN, LD], F32, isOutput=True)

    with tile.TileContext(nc) as tc:
        with (
            tc.tile_pool(name="wp", bufs=1) as wp,
            tc.tile_pool(name="cp", bufs=1) as cp,
            tc.tile_pool(name="ap", bufs=1) as ap_pool,
            tc.tile_pool(name="ep", bufs=3) as ep,
            tc.tile_pool(name="rp", bufs=2) as rp,
            tc.tile_pool(name="op", bufs=2) as op_pool,
            tc.tile_pool(name="ps", bufs=3, space="PSUM") as ps,
            tc.tile_pool(name="pss", bufs=1, space="PSUM") as pss,
        ):
            # ---- resident weights ----
            wvq_sb = [wp.tile([128, E], F16, tag=f"wvq{k}", name=f"wvq{k}") for k in range(8)]
            wvv_sb = [wp.tile([128, E], F16, tag=f"wvv{k}", name=f"wvv{k}") for k in range(8)]
            wlk_sb = [wp.tile([128, E], F16, tag=f"wlk{k}", name=f"wlk{k}") for k in range(6)]
            wlv_sb = [wp.tile([128, E], F16, tag=f"wlv{k}", name=f"wlv{k}") for k in range(6)]
            wvo_sb = [wp.tile([128, VD], F16, tag=f"wvo{k}", name=f"wvo{k}") for k in range(8)]
            wlo_sb = [wp.tile([128, LD], F16, tag=f"wlo{k}", name=f"wlo{k}") for k in range(8)]
            lt_sb = [wp.tile([128, BPC * LN], F16, tag=f"lt{k}", name=f"lt{k}") for k in range(6)]
            for k in range(8):
                nc.sync.dma_start(wvq_sb[k][:], wvq[k * 128:(k + 1) * 128, :])
                nc.sync.dma_start(wvv_sb[k][:], wvv[k * 128:(k + 1) * 128, :])
                nc.sync.dma_start(wvo_sb[k][:], wvo[k * 128:(k + 1) * 128, :])
                nc.sync.dma_start(wlo_sb[k][:], wlo[k * 128:(k + 1) * 128, :])
            for k in range(6):
                nc.sync.dma_start(wlk_sb[k][:], wlk[k * 128:(k + 1) * 128, :])
                nc.sync.dma_start(wlv_sb[k][:], wlv[k * 128:(k + 1) * 128, :])
                nc.sync.dma_start(lt_sb[k][:], lT[k * 128:(k + 1) * 128, :])
            bq = cp.tile([1, E], F16, tag="bq", name="bq")
            bk = cp.tile([1, E], F16, tag="bk", name="bk")
            bv = cp.tile([1, E], F16, tag="bv", name="bv")
            bl = cp.tile([1, E], F16, tag="bl", name="bl")
            bo_v = cp.tile([1, VD], F16, tag="bo_v", name="bo_v")
            bo_l = cp.tile([1, LD], F16, tag="bo_l", name="bo_l")
            nc.sync.dma_start(bq[:], bvq[:])
            nc.sync.dma_start(bk[:], blk[:])
            nc.sync.dma_start(bv[:], bvv[:])
            nc.sync.dma_start(bl[:], blv[:])
            nc.sync.dma_start(bo_v[:], bvo[:])
            nc.sync.dma_start(bo_l[:], blo[:])
            mb = cp.tile([LN, 1], F32, tag="mb", name="mb")
            nc.sync.dma_start(mb[:], maskb[:])
            m4 = cp.tile([128, 1], F32, tag="m4", name="m4")
            nc.vector.memset(m4[:], -SHIFT)

            # ---- constants ----
            ones = cp.tile([1, E], F16, tag="ones", name="ones")
            nc.vector.memset(ones[:], 1.0)
            ind2 = cp.tile([2, 128], F16, tag="ind2", name="ind2")
            oh = cp.tile([128, 2], F16, tag="oh", name="oh")
            oh2 = cp.tile([128, 2], F16, tag="oh2", name="oh2")
            nc.sync.dma_start(ind2[:], ind2_d[:])
            nc.sync.dma_start(oh[:], oh_d[:])
            nc.sync.dma_start(oh2[:], oh2_d[:])

            # ---- kT for all 4 batches:  kT[e, 4*128] ----
            kt = [wp.tile([128, BPC * LN], F16, tag=f"kt{m}", name=f"kt{m}") for m in range(8)]
            for m in range(8):
                acc = ps.tile([128, BPC * LN], F32, tag="ps", name="ps")
                for k in range(6):
                    nc.tensor.matmul(acc[:], wlk_sb[k][:, m * 128:(m + 1) * 128],
                                     lt_sb[k][:], start=(k == 0), stop=False)
                nc.tensor.matmul(acc[:], bk[0:1, m * 128:(m + 1) * 128],
                                 ones[0:1, 0:BPC * LN], start=False, stop=True)
                nc.vector.tensor_copy(kt[m][:], acc[:])

            # ---- lv (natural [s, e]) for all 4 batches ----
            lv = [[wp.tile([128, 512], F16, tag=f"lv{b}_{i}", name=f"lv{b}_{i}") for i in range(2)]
                  for b in range(BPC)]
            for b in range(BPC):
                for i, (n0, nn) in enumerate(N1024):
                    acc = ps.tile([128, 512], F32, tag="ps", name="ps")
                    for k in range(6):
                        nc.tensor.matmul(acc[0:128, 0:nn],
                                         lt_sb[k][:, b * 128:(b + 1) * 128],
                                         wlv_sb[k][:, n0:n0 + nn],
                                         start=(k == 0), stop=False)
                    nc.tensor.matmul(acc[0:128, 0:nn], ones[0:1, 0:128],
                                     bl[0:1, n0:n0 + nn], start=False, stop=True)
                    nc.vector.tensor_copy(lv[b][i][:], acc[0:128, 0:nn])

            # ---- per-batch ----
            for b in range(DBG_BPC or BPC):
                # load vT_b
                vt = [ap_pool.tile([128, VN], F16, tag=f"vt{k}", name=f"vt{k}") for k in range(8)]
                for k in range(8):
                    nc.sync.dma_start(vt[k][:], vT[b, k * 128:(k + 1) * 128, :])

                # qT_b [e, t] (scale folded into wvq/bvq on host)
                qt = [ap_pool.tile([128, VN], F16, tag=f"qt{m}", name=f"qt{m}") for m in range(8)]
                for m in range(8):
                    acc = ps.tile([128, VN], F32, tag="ps", name="ps")
                    for n0, nn in N576:
                        for k in range(8):
                            nc.tensor.matmul(acc[:, n0:n0 + nn],
                                             wvq_sb[k][:, m * 128:(m + 1) * 128],
                                             vt[k][:, n0:n0 + nn],
                                             start=(k == 0), stop=False)
                        nc.tensor.matmul(acc[:, n0:n0 + nn],
                                         bq[0:1, m * 128:(m + 1) * 128],
                                         ones[0:1, 0:nn], start=False, stop=True)
                    nc.vector.tensor_copy(qt[m][:], acc[:])

                # vv_b natural [t, e]
                vv = [ap_pool.tile([128, E], F16, tag=f"vv{c}", name=f"vv{c}") for c in range(5)]
                for c, (c0, cn) in enumerate(TC):
                    acc = ps.tile([128, E], F32, tag="ps", name="ps")
                    for n0, nn in N1024:
                        for k in range(8):
                            nc.tensor.matmul(acc[0:cn, n0:n0 + nn],
                                             vt[k][:, c0:c0 + cn],
                                             wvv_sb[k][:, n0:n0 + nn],
                                             start=(k == 0), stop=False)
                        nc.tensor.matmul(acc[0:cn, n0:n0 + nn], ones[0:1, 0:cn],
                                         bv[0:1, n0:n0 + nn], start=False, stop=True)
                    nc.vector.tensor_copy(vv[c][:cn, :], acc[0:cn, :])

                # attention, head pairs; per-pair sums, ACT log/exp reciprocal
                xvn = [ap_pool.tile([128, VN], F16, tag=f"xvn{j}", name=f"xvn{j}") for j in range(8)]
                xln = [ap_pool.tile([128, LN], F16, tag=f"xln{j}", name=f"xln{j}") for j in range(8)]
                for j in range(DBG_PAIRS or 8):
                    xv_u = ps.tile([128, 640], F32, tag="ps", name="xv_u")
                    xl_u = ps.tile([128, LN], F32, tag="ps", name="xl_u")
                    svl = pss.tile([2, 832], F32, tag="sum", name="svl")
                    for p in range(2):
                        h = 2 * j + p
                        hb = p * 64
                        ohp = oh if p == 0 else oh2
                        kth = kt[j][hb:hb + 64, b * 128:(b + 1) * 128]
                        qth = qt[j][hb:hb + 64, :]

                        # S_T [s, t] then E_v = exp(S_T + mask - SHIFT), fp16
                        st = ps.tile([128, 640], F32, tag="ps", name="st")
                        for n0, nn in N576:
                            nc.tensor.matmul(st[:, n0:n0 + nn], kth, qth[:, n0:n0 + nn],
                                             start=True, stop=True)
                        nc.vector.memset(st[:, VN:640], 0.0)
                        ev = ep.tile([128, 640], F16, tag="ev", name="ev")
                        nc.scalar.activation(ev[:], st[:], EXP, bias=mb[:], scale=1.0)

                        # S2 [t, s] then E2 = exp(S2 - SHIFT)  (no mask: attn_l)
                        s2 = ps.tile([128, 5 * 128], F32, tag="ps", name="s2")
                        for c, (c0, cn) in enumerate(TC):
                            nc.tensor.matmul(s2[0:cn, c * 128:(c + 1) * 128],
                                             qth[:, c0:c0 + cn], kth,
                                             start=True, stop=True)
                        e2 = ep.tile([128, 5 * 128], F16, tag="e2", name="e2")
                        nc.scalar.activation(e2[:], s2[:], EXP, bias=m4[:], scale=1.0)

                        # sumexp rows (bank-window discipline: one clearer per bank)
                        nc.tensor.matmul(svl[0:2, 0:512], ohp, ev[:, 0:512],
                                         start=(p == 0), stop=(p == 1),
                                         skip_group_check=True)
                        mm_sv2 = nc.tensor.matmul(svl[0:2, 576:704], ohp, ev[:, 512:640],
                                                  start=(p == 0), stop=False,
                                                  skip_group_check=True)
                        if p == 0:
                            sv2_first = mm_sv2
                        for c, (c0, cn) in enumerate(TC):
                            mm_sl = nc.tensor.matmul(svl[0:2, 704:704 + LN], ohp[0:cn, :],
                                                     e2[0:cn, c * 128:(c + 1) * 128],
                                                     start=False,
                                                     stop=(p == 1 and c == 4),
                                                     skip_group_check=True)
                            add_dep_helper(mm_sl.ins, sv2_first.ins, sync=False,
                                           reason="bank7 window: clear-first")

                        # PV_v: xv_u[hb:hb+64, t] = lv_h^T(as lhsT) @ E_v
                        lvh = lv[b][h // 8][:, (h % 8) * 64:(h % 8) * 64 + 64]
                        for n0, nn in [(0, 512), (512, 128)]:
                            nc.tensor.matmul(xv_u[hb:hb + 64, n0:n0 + nn], lvh,
                                             ev[:, n0:n0 + nn], start=True, stop=True)
                        # PV_l: xl_u[hb:hb+64, s] = sum_c vv_h_c^T @ E2_c
                        for c, (c0, cn) in enumerate(TC):
                            mm_xl = nc.tensor.matmul(xl_u[hb:hb + 64, :],
                                                     vv[c][0:cn, h * 64:h * 64 + 64],
                                                     e2[0:cn, c * 128:(c + 1) * 128],
                                                     start=(p == 0 and c == 0),
                                                     stop=(p == 1 and c == 4),
                                                     skip_group_check=True)
                            if p == 0 and c == 0:
                                xl_first = mm_xl
                            else:
                                add_dep_helper(mm_xl.ins, xl_first.ins, sync=False,
                                               reason="xl_u bank window: clear-first")

                    # reciprocal of sums on ACT: rcp = exp(-log(sums)), f16 out
                    lgt = rp.tile([2, 832], F32, tag="lgt", name="lgt")
                    nc.scalar.activation(lgt[:], svl[0:2, 0:832], LOG, bias=0.0, scale=1.0)
                    rcp = rp.tile([2, 832], F16, tag="rcp", name="rcp")
                    nc.scalar.activation(rcp[:], lgt[:], EXP, bias=0.0, scale=-1.0)

                    bcv = ps.tile([128, 640], F32, tag="ps", name="bcv")
                    nc.tensor.matmul(bcv[:, 0:512], ind2[:], rcp[:, 0:512],
                                     start=True, stop=True)
                    nc.tensor.matmul(bcv[:, 512:576], ind2[:], rcp[:, 576:640],
                                     start=True, stop=True)
                    bcv_sb = rp.tile([128, VN], F16, tag="bcv_sb", name="bcv_sb")
                    nc.vector.tensor_copy(bcv_sb[:], bcv[:, 0:VN])
                    nc.vector.tensor_tensor(xvn[j][:], xv_u[:, 0:VN], bcv_sb[:],
                                            op=mybir.AluOpType.mult)
                    bcl = ps.tile([128, LN], F32, tag="ps", name="bcl")
                    nc.tensor.matmul(bcl[:], ind2[:], rcp[:, 704:832], start=True, stop=True)
                    bcl_sb = rp.tile([128, LN], F16, tag="bcl_sb", name="bcl_sb")
                    nc.vector.tensor_copy(bcl_sb[:], bcl[:])
                    nc.vector.tensor_tensor(xln[j][:], xl_u[:], bcl_sb[:],
                                            op=mybir.AluOpType.mult)

                # xv out-projection -> natural [t, vd]
                if DBG_PAIRS:
                    continue
                for c, (c0, cn) in enumerate(TC):
                    oev = op_pool.tile([128, VD], F32, tag="oev", name="oev")
                    for n0, nn in N1024:
                        acc = ps.tile([128, 512], F32, tag="ps", name="ps")
                        for jj in range(8):
                            nc.tensor.matmul(acc[0:cn, 0:nn],
                                             xvn[jj][:, c0:c0 + cn],
                                             wvo_sb[jj][:, n0:n0 + nn],
                                             start=(jj == 0), stop=False)
                        nc.tensor.matmul(acc[0:cn, 0:nn], ones[0:1, 0:cn],
                                         bo_v[0:1, n0:n0 + nn], start=False, stop=True)
                        nc.scalar.copy(oev[0:cn, n0:n0 + nn], acc[0:cn, 0:nn])
                    nc.sync.dma_start(xv[b, c0:c0 + cn, :], oev[0:cn, :])

                # xl out-projection -> natural [s, ld]
                oel = op_pool.tile([128, LD], F32, tag="oel", name="oel")
                for n0, nn in N768:
                    acc = ps.tile([128, 512], F32, tag="ps", name="ps")
                    for jj in range(8):
                        nc.tensor.matmul(acc[:, 0:nn], xln[jj][:],
                                         wlo_sb[jj][:, n0:n0 + nn],
                                         start=(jj == 0), stop=False)
                    nc.tensor.matmul(acc[:, 0:nn], ones[0:1, 0:128],
                                     bo_l[0:1, n0:n0 + nn], start=False, stop=True)
                    nc.scalar.copy(oel[:, n0:n0 + nn], acc[:, 0:nn])
                nc.sync.dma_start(xl[b][:], oel[:])

    nc.compile()
    return nc


def kernel(v, l, attention_mask_l, Wvq, bvq, Wlk, blk, Wvv, bvv, Wlv, blv,
           Wvo, bvo, Wlo, blo):
    global _cached_nc
    from concourse.bass_utils import run_bass_kernel_spmd

    if _cached_nc is None:
        _cached_nc = _build_nc()
    nc = _cached_nc

    f16 = np.float16
    v = np.asarray(v, dtype=np.float32)
    l = np.asarray(l, dtype=np.float32)
    shared = {
        "wvq": np.ascontiguousarray((SCALE * np.asarray(Wvq, np.float32).T).astype(f16)),
        "wlk": np.ascontiguousarray(np.asarray(Wlk, np.float32).T.astype(f16)),
        "wvv": np.ascontiguousarray(np.asarray(Wvv, np.float32).T.astype(f16)),
        "wlv": np.ascontiguousarray(np.asarray(Wlv, np.float32).T.astype(f16)),
        "wvo": np.ascontiguousarray(np.asarray(Wvo, np.float32).T.astype(f16)),
        "wlo": np.ascontiguousarray(np.asarray(Wlo, np.float32).T.astype(f16)),
        "bvq": (SCALE * np.asarray(bvq, np.float32)).reshape(1, E).astype(f16),
        "blk": np.asarray(blk, np.float32).reshape(1, E).astype(f16),
        "bvv": np.asarray(bvv, np.float32).reshape(1, E).astype(f16),
        "blv": np.asarray(blv, np.float32).reshape(1, E).astype(f16),
        "bvo": np.asarray(bvo, np.float32).reshape(1, VD).astype(f16),
        "blo": np.asarray(blo, np.float32).reshape(1, LD).astype(f16),
        "maskb": (np.asarray(attention_mask_l, np.float32) - SHIFT)
                 .reshape(LN, 1).astype(np.float32),
        "ind2": np.kron(np.eye(2), np.ones((1, 64))).astype(f16),
        "oh": np.stack([np.ones(128), np.zeros(128)], axis=1).astype(f16),
        "oh2": np.stack([np.zeros(128), np.ones(128)], axis=1).astype(f16),
    }
    in_maps = []
    for c in range(NCORES):
        vc = v[c * BPC:(c + 1) * BPC]
        lc = l[c * BPC:(c + 1) * BPC]
        m = dict(shared)
        m["vT"] = np.ascontiguousarray(vc.transpose(0, 2, 1)).astype(f16)
        m["lT"] = np.ascontiguousarray(lc.transpose(2, 0, 1).reshape(LD, BPC * LN)).astype(f16)
        in_maps.append(m)

    global _last_in_maps
    _last_in_maps = in_maps
    res = run_bass_kernel_spmd(nc, in_maps, list(range(NCORES)))
    global _last_res
    _last_res = res.results[0]
    xv = np.concatenate([res.results[c]["xv"] for c in range(NCORES)], axis=0)
    xl = np.concatenate([res.results[c]["xl"] for c in range(NCORES)], axis=0)
    return xv, xl


# revision 28
# speedup vs baseline: 1.1885x; 1.1885x over previous
"""BiMultiHeadAttention (vision-language cross attention) on 8 Trainium2 cores.

Strategy: pure data-parallel over batch (32 batches -> 4 per core).
All matmul data in fp16 (11-bit mantissa; errors ~1e-3 on the logit path),
accumulation in fp32 PSUM.  Layout chain keeps the contraction axis on
partitions at every step:

  vT/lT (host-pretransposed, fp16)
    -> qT,kT in [e, t] layout;  vv,lv in natural [t, e] layout
    -> S_T[s,t] and S2[t,s] both by matmul (bidirectional softmax needs both)
    -> E = exp(S - 4) in fp16 (shift keeps exp in fp16 range; softmax-invariant)
    -> PV matmuls produce un-normalized xv^T[e,t], xl^T[e,s] per head pair
    -> per-pair sumexp rows via one-hot matmuls; reciprocal; K=2 broadcast
       matmul builds the per-(head,column) scale tile; DVE multiply normalizes
    -> output projections back to natural layout, bias added as K=1 matmul rows.

The attention mask enters E_v as a per-partition bias (exact for any mask);
attn_l/softmax-over-queries is mask-free per the reference.
"""
import sys
import numpy as np

sys.path.insert(0, "/opt/trn_rl_repo")

B, VN, LN = 32, 576, 128
E, H, D = 1024, 16, 64
VD, LD = 1024, 768
NCORES = 8
BPC = B // NCORES          # batches per core
SCALE = H ** -0.5          # 0.25 (reference scales by num_heads**-0.5)
SHIFT = 4.0                # exp(logit - SHIFT) keeps E in fp16 range
TC = [(0, 128), (128, 128), (256, 128), (384, 128), (512, 64)]   # t chunks of 576
N576 = [(0, 512), (512, 64)]
N1024 = [(0, 512), (512, 512)]
N768 = [(0, 512), (512, 256)]

_cached_nc = None


def _oh16():
    m = np.zeros((128, 256), np.float16)
    for h in range(16):
        m[:, h * 16 + h] = 1.0
    return m


def _ind16():
    m = np.zeros((8, 16, 128), np.float16)
    for j in range(8):
        m[j, 2 * j, 0:64] = 1.0
        m[j, 2 * j + 1, 64:128] = 1.0
    return m
import os as _os
DBG_BPC = int(_os.environ.get("DBG_BPC", "0")) or None   # limit batches built
DBG_PAIRS = int(_os.environ.get("DBG_PAIRS", "0")) or None


def _build_nc():
    import concourse.bacc as bacc
    import concourse.hw_specs as hw_specs
    # Prefer the combined ln+exp ACT table set so alternating Ln/Exp
    # activations don't thrash ACT_TABLE_LOAD (~1.3us per reload).
    if not getattr(hw_specs, "_combined_ln_exp_first", False):
        _orig_get_tables = hw_specs.get_activation_tables

        def _reordered(arch):
            t = dict(_orig_get_tables(arch))   # order == act_info.json index order
            key = "natural_log_exp_and_others"
            if key in t:
                shared = t[key]
                strip = {f for f in shared}
                out = {}
                for k, v in t.items():
                    if k == key:
                        out[k] = v
                    else:
                        out[k] = v - strip if isinstance(v, set) else v
                return out
            return t

        hw_specs.get_activation_tables = _reordered
        bacc.get_activation_tables = _reordered
        hw_specs._combined_ln_exp_first = True
    import concourse.mybir as mybir
    import concourse.tile as tile
    from concourse.tile import add_dep_helper

    F16 = mybir.dt.float16
    F32 = mybir.dt.float32
    EXP = mybir.ActivationFunctionType.Exp
    LOG = mybir.ActivationFunctionType.Ln

    nc = bacc.Bacc("TRN2", target_bir_lowering=False, debug=False)
    vT = nc.declare_dram_parameter("vT", [BPC, VD, VN], F16, isOutput=False)
    lT = nc.declare_dram_parameter("lT", [LD, BPC * LN], F16, isOutput=False)
    wvq = nc.declare_dram_parameter("wvq", [VD, E], F16, isOutput=False)
    wlk = nc.declare_dram_parameter("wlk", [LD, E], F16, isOutput=False)
    wvv = nc.declare_dram_parameter("wvv", [VD, E], F16, isOutput=False)
    wlv = nc.declare_dram_parameter("wlv", [LD, E], F16, isOutput=False)
    wvo = nc.declare_dram_parameter("wvo", [E, VD], F16, isOutput=False)
    wlo = nc.declare_dram_parameter("wlo", [E, LD], F16, isOutput=False)
    bvq = nc.declare_dram_parameter("bvq", [1, E], F16, isOutput=False)
    blk = nc.declare_dram_parameter("blk", [1, E], F16, isOutput=False)
    bvv = nc.declare_dram_parameter("bvv", [1, E], F16, isOutput=False)
    blv = nc.declare_dram_parameter("blv", [1, E], F16, isOutput=False)
    bvo = nc.declare_dram_parameter("bvo", [1, VD], F16, isOutput=False)
    blo = nc.declare_dram_parameter("blo", [1, LD], F16, isOutput=False)
    maskb = nc.declare_dram_parameter("maskb", [LN, 1], F32, isOutput=False)
    ind2_d = nc.declare_dram_parameter("ind2", [2, 128], F16, isOutput=False)
    oh_d = nc.declare_dram_parameter("oh", [128, 2], F16, isOutput=False)
    oh2_d = nc.declare_dram_parameter("oh2", [128, 2], F16, isOutput=False)
    xv = nc.declare_dram_parameter("xv", [BPC, VN, VD], F32, isOutput=True)
    xl = nc.declare_dram_parameter("xl", [BPC, LN, LD], F32, isOutput=True)

    with tile.TileContext(nc) as tc:
        with (
            tc.tile_pool(name="wp", bufs=1) as wp,
            tc.tile_pool(name="cp", bufs=1) as cp,
            tc.tile_pool(name="ap", bufs=1) as ap_pool,
            tc.tile_pool(name="ep", bufs=4) as ep,
            tc.tile_pool(name="rp", bufs=2) as rp,
            tc.tile_pool(name="op", bufs=2) as op_pool,
            tc.tile_pool(name="ps", bufs=3, space="PSUM") as ps,
            tc.tile_pool(name="pss", bufs=1, space="PSUM") as pss,
        ):
            # ---- resident weights ----
            wvq_sb = [wp.tile([128, E], F16, tag=f"wvq{k}", name=f"wvq{k}") for k in range(8)]
            wvv_sb = [wp.tile([128, E], F16, tag=f"wvv{k}", name=f"wvv{k}") for k in range(8)]
            wlk_sb = [wp.tile([128, E], F16, tag=f"wlk{k}", name=f"wlk{k}") for k in range(6)]
            wlv_sb = [wp.tile([128, E], F16, tag=f"wlv{k}", name=f"wlv{k}") for k in range(6)]
            wvo_sb = [wp.tile([128, VD], F16, tag=f"wvo{k}", name=f"wvo{k}") for k in range(8)]
            wlo_sb = [wp.tile([128, LD], F16, tag=f"wlo{k}", name=f"wlo{k}") for k in range(8)]
            lt_sb = [wp.tile([128, BPC * LN], F16, tag=f"lt{k}", name=f"lt{k}") for k in range(6)]
            for k in range(8):
                nc.sync.dma_start(wvq_sb[k][:], wvq[k * 128:(k + 1) * 128, :])
                nc.sync.dma_start(wvv_sb[k][:], wvv[k * 128:(k + 1) * 128, :])
                nc.sync.dma_start(wvo_sb[k][:], wvo[k * 128:(k + 1) * 128, :])
                nc.sync.dma_start(wlo_sb[k][:], wlo[k * 128:(k + 1) * 128, :])
            for k in range(6):
                nc.sync.dma_start(wlk_sb[k][:], wlk[k * 128:(k + 1) * 128, :])
                nc.sync.dma_start(wlv_sb[k][:], wlv[k * 128:(k + 1) * 128, :])
                nc.sync.dma_start(lt_sb[k][:], lT[k * 128:(k + 1) * 128, :])
            bq = cp.tile([1, E], F16, tag="bq", name="bq")
            bk = cp.tile([1, E], F16, tag="bk", name="bk")
            bv = cp.tile([1, E], F16, tag="bv", name="bv")
            bl = cp.tile([1, E], F16, tag="bl", name="bl")
            bo_v = cp.tile([1, VD], F16, tag="bo_v", name="bo_v")
            bo_l = cp.tile([1, LD], F16, tag="bo_l", name="bo_l")
            nc.sync.dma_start(bq[:], bvq[:])
            nc.sync.dma_start(bk[:], blk[:])
            nc.sync.dma_start(bv[:], bvv[:])
            nc.sync.dma_start(bl[:], blv[:])
            nc.sync.dma_start(bo_v[:], bvo[:])
            nc.sync.dma_start(bo_l[:], blo[:])
            mb = cp.tile([LN, 1], F32, tag="mb", name="mb")
            nc.sync.dma_start(mb[:], maskb[:])
            m4 = cp.tile([128, 1], F32, tag="m4", name="m4")
            nc.vector.memset(m4[:], -SHIFT)

            # ---- constants ----
            ones = cp.tile([1, E], F16, tag="ones", name="ones")
            nc.vector.memset(ones[:], 1.0)
            ind2 = cp.tile([2, 128], F16, tag="ind2", name="ind2")
            oh = cp.tile([128, 2], F16, tag="oh", name="oh")
            oh2 = cp.tile([128, 2], F16, tag="oh2", name="oh2")
            nc.sync.dma_start(ind2[:], ind2_d[:])
            nc.sync.dma_start(oh[:], oh_d[:])
            nc.sync.dma_start(oh2[:], oh2_d[:])

            # ---- kT for all 4 batches:  kT[e, 4*128] ----
            kt = [wp.tile([128, BPC * LN], F16, tag=f"kt{m}", name=f"kt{m}") for m in range(8)]
            for m in range(8):
                acc = ps.tile([128, BPC * LN], F32, tag="ps", name="ps")
                for k in range(6):
                    nc.tensor.matmul(acc[:], wlk_sb[k][:, m * 128:(m + 1) * 128],
                                     lt_sb[k][:], start=(k == 0), stop=False)
                nc.tensor.matmul(acc[:], bk[0:1, m * 128:(m + 1) * 128],
                                 ones[0:1, 0:BPC * LN], start=False, stop=True)
                nc.vector.tensor_copy(kt[m][:], acc[:])

            # ---- lv (natural [s, e]) for all 4 batches ----
            lv = [[wp.tile([128, 512], F16, tag=f"lv{b}_{i}", name=f"lv{b}_{i}") for i in range(2)]
                  for b in range(BPC)]
            for b in range(BPC):
                for i, (n0, nn) in enumerate(N1024):
                    acc = ps.tile([128, 512], F32, tag="ps", name="ps")
                    for k in range(6):
                        nc.tensor.matmul(acc[0:128, 0:nn],
                                         lt_sb[k][:, b * 128:(b + 1) * 128],
                                         wlv_sb[k][:, n0:n0 + nn],
                                         start=(k == 0), stop=False)
                    nc.tensor.matmul(acc[0:128, 0:nn], ones[0:1, 0:128],
                                     bl[0:1, n0:n0 + nn], start=False, stop=True)
                    nc.vector.tensor_copy(lv[b][i][:], acc[0:128, 0:nn])

            # ---- per-batch ----
            for b in range(DBG_BPC or BPC):
                # load vT_b
                vt = [ap_pool.tile([128, VN], F16, tag=f"vt{k}", name=f"vt{k}") for k in range(8)]
                for k in range(8):
                    nc.sync.dma_start(vt[k][:], vT[b, k * 128:(k + 1) * 128, :])

                # qT_b [e, t] (scale folded into wvq/bvq on host)
                qt = [ap_pool.tile([128, VN], F16, tag=f"qt{m}", name=f"qt{m}") for m in range(8)]
                for m in range(8):
                    acc = ps.tile([128, VN], F32, tag="ps", name="ps")
                    for n0, nn in N576:
                        for k in range(8):
                            nc.tensor.matmul(acc[:, n0:n0 + nn],
                                             wvq_sb[k][:, m * 128:(m + 1) * 128],
                                             vt[k][:, n0:n0 + nn],
                                             start=(k == 0), stop=False)
                        nc.tensor.matmul(acc[:, n0:n0 + nn],
                                         bq[0:1, m * 128:(m + 1) * 128],
                                         ones[0:1, 0:nn], start=False, stop=True)
                    nc.vector.tensor_copy(qt[m][:], acc[:])

                # vv_b natural [t, e]
                vv = [ap_pool.tile([128, E], F16, tag=f"vv{c}", name=f"vv{c}") for c in range(5)]
                for c, (c0, cn) in enumerate(TC):
                    acc = ps.tile([128, E], F32, tag="ps", name="ps")
                    for n0, nn in N1024:
                        for k in range(8):
                            nc.tensor.matmul(acc[0:cn, n0:n0 + nn],
                                             vt[k][:, c0:c0 + cn],
                                             wvv_sb[k][:, n0:n0 + nn],
                                             start=(k == 0), stop=False)
                        nc.tensor.matmul(acc[0:cn, n0:n0 + nn], ones[0:1, 0:cn],
                                         bv[0:1, n0:n0 + nn], start=False, stop=True)
                    nc.vector.tensor_copy(vv[c][:cn, :], acc[0:cn, :])

                # attention, head pairs; per-pair sums, ACT log/exp reciprocal
                xvn = [ap_pool.tile([128, VN], F16, tag=f"xvn{j}", name=f"xvn{j}") for j in range(8)]
                xln = [ap_pool.tile([128, LN], F16, tag=f"xln{j}", name=f"xln{j}") for j in range(8)]
                for j in range(DBG_PAIRS or 8):
                    xv_u = ps.tile([128, 640], F32, tag="ps", name="xv_u")
                    xl_u = ps.tile([128, LN], F32, tag="ps", name="xl_u")
                    svl = pss.tile([2, 832], F32, tag="sum", name="svl")
                    for p in range(2):
                        h = 2 * j + p
                        hb = p * 64
                        ohp = oh if p == 0 else oh2
                        kth = kt[j][hb:hb + 64, b * 128:(b + 1) * 128]
                        qth = qt[j][hb:hb + 64, :]

                        # S_T [s, t] then E_v = exp(S_T + mask - SHIFT), fp16
                        st = ps.tile([128, 640], F32, tag="ps", name="st")
                        for n0, nn in N576:
                            nc.tensor.matmul(st[:, n0:n0 + nn], kth, qth[:, n0:n0 + nn],
                                             start=True, stop=True)
                        nc.vector.memset(st[:, VN:640], 0.0)
                        ev = ep.tile([128, 640], F16, tag="ev", name="ev")
                        nc.scalar.activation(ev[:], st[:], EXP, bias=mb[:], scale=1.0)

                        # S2 [t, s] then E2 = exp(S2 - SHIFT)  (no mask: attn_l)
                        s2 = ps.tile([128, 5 * 128], F32, tag="ps", name="s2")
                        for c, (c0, cn) in enumerate(TC):
                            nc.tensor.matmul(s2[0:cn, c * 128:(c + 1) * 128],
                                             qth[:, c0:c0 + cn], kth,
                                             start=True, stop=True)
                        e2 = ep.tile([128, 5 * 128], F16, tag="e2", name="e2")
                        nc.scalar.activation(e2[:], s2[:], EXP, bias=m4[:], scale=1.0)

                        # sumexp rows (bank-window discipline: one clearer per bank)
                        nc.tensor.matmul(svl[0:2, 0:512], ohp, ev[:, 0:512],
                                         start=(p == 0), stop=(p == 1),
                                         skip_group_check=True)
                        mm_sv2 = nc.tensor.matmul(svl[0:2, 576:704], ohp, ev[:, 512:640],
                                                  start=(p == 0), stop=False,
                                                  skip_group_check=True)
                        if p == 0:
                            sv2_first = mm_sv2
                        for c, (c0, cn) in enumerate(TC):
                            mm_sl = nc.tensor.matmul(svl[0:2, 704:704 + LN], ohp[0:cn, :],
                                                     e2[0:cn, c * 128:(c + 1) * 128],
                                                     start=False,
                                                     stop=(p == 1 and c == 4),
                                                     skip_group_check=True)
                            add_dep_helper(mm_sl.ins, sv2_first.ins, sync=False,
                                           reason="bank7 window: clear-first")

                        # PV_v: xv_u[hb:hb+64, t] = lv_h^T(as lhsT) @ E_v
                        lvh = lv[b][h // 8][:, (h % 8) * 64:(h % 8) * 64 + 64]
                        for n0, nn in [(0, 512), (512, 128)]:
                            nc.tensor.matmul(xv_u[hb:hb + 64, n0:n0 + nn], lvh,
                                             ev[:, n0:n0 + nn], start=True, stop=True)
                        # PV_l: xl_u[hb:hb+64, s] = sum_c vv_h_c^T @ E2_c
                        for c, (c0, cn) in enumerate(TC):
                            mm_xl = nc.tensor.matmul(xl_u[hb:hb + 64, :],
                                                     vv[c][0:cn, h * 64:h * 64 + 64],
                                                     e2[0:cn, c * 128:(c + 1) * 128],
                                                     start=(p == 0 and c == 0),
                                                     stop=(p == 1 and c == 4),
                                                     skip_group_check=True)
                            if p == 0 and c == 0:
                                xl_first = mm_xl
                            else:
                                add_dep_helper(mm_xl.ins, xl_first.ins, sync=False,
                                               reason="xl_u bank window: clear-first")

                    # reciprocal of sums on ACT: rcp = exp(-log(sums)), f16 out
                    lgt = rp.tile([2, 832], F32, tag="lgt", name="lgt")
                    nc.scalar.activation(lgt[:], svl[0:2, 0:832], LOG, bias=0.0, scale=1.0)
                    rcp = rp.tile([2, 832], F16, tag="rcp", name="rcp")
                    nc.scalar.activation(rcp[:], lgt[:], EXP, bias=0.0, scale=-1.0)

                    bcv = ps.tile([128, 640], F32, tag="ps", name="bcv")
                    nc.tensor.matmul(bcv[:, 0:512], ind2[:], rcp[:, 0:512],
                                     start=True, stop=True)
                    nc.tensor.matmul(bcv[:, 512:576], ind2[:], rcp[:, 576:640],
                                     start=True, stop=True)
                    bcv_sb = rp.tile([128, VN], F16, tag="bcv_sb", name="bcv_sb")
                    nc.vector.tensor_copy(bcv_sb[:], bcv[:, 0:VN])
                    nc.vector.tensor_tensor(xvn[j][:], xv_u[:, 0:VN], bcv_sb[:],
                                            op=mybir.AluOpType.mult)
                    bcl = ps.tile([128, LN], F32, tag="ps", name="bcl")
                    nc.tensor.matmul(bcl[:], ind2[:], rcp[:, 704:832], start=True, stop=True)
                    bcl_sb = rp.tile([128, LN], F16, tag="bcl_sb", name="bcl_sb")
                    nc.vector.tensor_copy(bcl_sb[:], bcl[:])
                    nc.vector.tensor_tensor(xln[j][:], xl_u[:], bcl_sb[:],
                                            op=mybir.AluOpType.mult)

                # xv out-projection -> natural [t, vd]
                if DBG_PAIRS:
                    continue
                for c, (c0, cn) in enumerate(TC):
                    oev = op_pool.tile([128, VD], F32, tag="oev", name="oev")
                    for n0, nn in N1024:
                        acc = ps.tile([128, 512], F32, tag="ps", name="ps")
                        for jj in range(8):
                            nc.tensor.matmul(acc[0:cn, 0:nn],
                                             xvn[jj][:, c0:c0 + cn],
                                             wvo_sb[jj][:, n0:n0 + nn],
                                             start=(jj == 0), stop=False)
                        nc.tensor.matmul(acc[0:cn, 0:nn], ones[0:1, 0:cn],
                                         bo_v[0:1, n0:n0 + nn], start=False, stop=True)
                        nc.scalar.copy(oev[0:cn, n0:n0 + nn], acc[0:cn, 0:nn])
                    nc.sync.dma_start(xv[b, c0:c0 + cn, :], oev[0:cn, :])

                # xl out-projection -> natural [s, ld]
                oel = op_pool.tile([128, LD], F32, tag="oel", name="oel")
                for n0, nn in N768:
                    acc = ps.tile([128, 512], F32, tag="ps", name="ps")
                    for jj in range(8):
                        nc.tensor.matmul(acc[:, 0:nn], xln[jj][:],
                                         wlo_sb[jj][:, n0:n0 + nn],
                                         start=(jj == 0), stop=False)
                    nc.tensor.matmul(acc[:, 0:nn], ones[0:1, 0:128],
                                     bo_l[0:1, n0:n0 + nn], start=False, stop=True)
                    nc.scalar.copy(oel[:, n0:n0 + nn], acc[:, 0:nn])
                nc.sync.dma_start(xl[b][:], oel[:])

    nc.compile()
    return nc


def kernel(v, l, attention_mask_l, Wvq, bvq, Wlk, blk, Wvv, bvv, Wlv, blv,
           Wvo, bvo, Wlo, blo):
    global _cached_nc
    from concourse.bass_utils import run_bass_kernel_spmd

    if _cached_nc is None:
        _cached_nc = _build_nc()
    nc = _cached_nc

    f16 = np.float16
    v = np.asarray(v, dtype=np.float32)
    l = np.asarray(l, dtype=np.float32)
    shared = {
        "wvq": np.ascontiguousarray((SCALE * np.asarray(Wvq, np.float32).T).astype(f16)),
        "wlk": np.ascontiguousarray(np.asarray(Wlk, np.float32).T.astype(f16)),
        "wvv": np.ascontiguousarray(np.asarray(Wvv, np.float32).T.astype(f16)),
        "wlv": np.ascontiguousarray(np.asarray(Wlv, np.float32).T.astype(f16)),
        "wvo": np.ascontiguousarray(np.asarray(Wvo, np.float32).T.astype(f16)),
        "wlo": np.ascontiguousarray(np.asarray(Wlo, np.float32).T.astype(f16)),
        "bvq": (SCALE * np.asarray(bvq, np.float32)).reshape(1, E).astype(f16),
        "blk": np.asarray(blk, np.float32).reshape(1, E).astype(f16),
        "bvv": np.asarray(bvv, np.float32).reshape(1, E).astype(f16),
        "blv": np.asarray(blv, np.float32).reshape(1, E).astype(f16),
        "bvo": np.asarray(bvo, np.float32).reshape(1, VD).astype(f16),
        "blo": np.asarray(blo, np.float32).reshape(1, LD).astype(f16),
        "maskb": (np.asarray(attention_mask_l, np.float32) - SHIFT)
                 .reshape(LN, 1).astype(np.float32),
        "ind2": np.kron(np.eye(2), np.ones((1, 64))).astype(f16),
        "oh": np.stack([np.ones(128), np.zeros(128)], axis=1).astype(f16),
        "oh2": np.stack([np.zeros(128), np.ones(128)], axis=1).astype(f16),
    }
    in_maps = []
    for c in range(NCORES):
        vc = v[c * BPC:(c + 1) * BPC]
        lc = l[c * BPC:(c + 1) * BPC]
        m = dict(shared)
        m["vT"] = np.ascontiguousarray(vc.transpose(0, 2, 1)).astype(f16)
        m["lT"] = np.ascontiguousarray(lc.transpose(2, 0, 1).reshape(LD, BPC * LN)).astype(f16)
        in_maps.append(m)

    global _last_in_maps
    _last_in_maps = in_maps
    res = run_bass_kernel_spmd(nc, in_maps, list(range(NCORES)))
    global _last_res
    _last_res = res.results[0]
    xv = np.concatenate([res.results[c]["xv"] for c in range(NCORES)], axis=0)
    xl = np.concatenate([res.results[c]["xl"] for c in range(NCORES)], axis=0)
    return xv, xl


# revision 30
# speedup vs baseline: 1.1918x; 1.0028x over previous
"""BiMultiHeadAttention (vision-language cross attention) on 8 Trainium2 cores.

Strategy: pure data-parallel over batch (32 batches -> 4 per core).
All matmul data in fp16 (11-bit mantissa; errors ~1e-3 on the logit path),
accumulation in fp32 PSUM.  Layout chain keeps the contraction axis on
partitions at every step:

  vT/lT (host-pretransposed, fp16)
    -> qT,kT in [e, t] layout;  vv,lv in natural [t, e] layout
    -> S_T[s,t] and S2[t,s] both by matmul (bidirectional softmax needs both)
    -> E = exp(S - 4) in fp16 (shift keeps exp in fp16 range; softmax-invariant)
    -> PV matmuls produce un-normalized xv^T[e,t], xl^T[e,s] per head pair
    -> per-pair sumexp rows via one-hot matmuls; reciprocal; K=2 broadcast
       matmul builds the per-(head,column) scale tile; DVE multiply normalizes
    -> output projections back to natural layout, bias added as K=1 matmul rows.

The attention mask enters E_v as a per-partition bias (exact for any mask);
attn_l/softmax-over-queries is mask-free per the reference.
"""
import sys
import numpy as np

sys.path.insert(0, "/opt/trn_rl_repo")

B, VN, LN = 32, 576, 128
E, H, D = 1024, 16, 64
VD, LD = 1024, 768
NCORES = 8
BPC = B // NCORES          # batches per core
SCALE = H ** -0.5          # 0.25 (reference scales by num_heads**-0.5)
SHIFT = 4.0                # exp(logit - SHIFT) keeps E in fp16 range
TC = [(0, 128), (128, 128), (256, 128), (384, 128), (512, 64)]   # t chunks of 576
N576 = [(0, 512), (512, 64)]
N1024 = [(0, 512), (512, 512)]
N768 = [(0, 512), (512, 256)]

_cached_nc = None


def _oh16():
    m = np.zeros((128, 256), np.float16)
    for h in range(16):
        m[:, h * 16 + h] = 1.0
    return m


def _ind16():
    m = np.zeros((8, 16, 128), np.float16)
    for j in range(8):
        m[j, 2 * j, 0:64] = 1.0
        m[j, 2 * j + 1, 64:128] = 1.0
    return m
import os as _os
DBG_BPC = int(_os.environ.get("DBG_BPC", "0")) or None   # limit batches built
DBG_PAIRS = int(_os.environ.get("DBG_PAIRS", "0")) or None


def _build_nc():
    import concourse.bacc as bacc
    import concourse.hw_specs as hw_specs
    # Prefer the combined ln+exp ACT table set so alternating Ln/Exp
    # activations don't thrash ACT_TABLE_LOAD (~1.3us per reload).
    if not getattr(hw_specs, "_combined_ln_exp_first", False):
        _orig_get_tables = hw_specs.get_activation_tables

        def _reordered(arch):
            t = dict(_orig_get_tables(arch))   # order == act_info.json index order
            key = "natural_log_exp_and_others"
            if key in t:
                shared = t[key]
                strip = {f for f in shared}
                out = {}
                for k, v in t.items():
                    if k == key:
                        out[k] = v
                    else:
                        out[k] = v - strip if isinstance(v, set) else v
                return out
            return t

        hw_specs.get_activation_tables = _reordered
        bacc.get_activation_tables = _reordered
        hw_specs._combined_ln_exp_first = True
    import concourse.mybir as mybir
    import concourse.tile as tile
    from concourse.tile import add_dep_helper

    F16 = mybir.dt.float16
    F32 = mybir.dt.float32
    EXP = mybir.ActivationFunctionType.Exp
    LOG = mybir.ActivationFunctionType.Ln

    nc = bacc.Bacc("TRN2", target_bir_lowering=False, debug=False)
    vT = nc.declare_dram_parameter("vT", [BPC, VD, VN], F16, isOutput=False)
    lT = nc.declare_dram_parameter("lT", [LD, BPC * LN], F16, isOutput=False)
    wvq = nc.declare_dram_parameter("wvq", [VD, E], F16, isOutput=False)
    wlk = nc.declare_dram_parameter("wlk", [LD, E], F16, isOutput=False)
    wvv = nc.declare_dram_parameter("wvv", [VD, E], F16, isOutput=False)
    wlv = nc.declare_dram_parameter("wlv", [LD, E], F16, isOutput=False)
    wvo = nc.declare_dram_parameter("wvo", [E, VD], F16, isOutput=False)
    wlo = nc.declare_dram_parameter("wlo", [E, LD], F16, isOutput=False)
    bvq = nc.declare_dram_parameter("bvq", [1, E], F16, isOutput=False)
    blk = nc.declare_dram_parameter("blk", [1, E], F16, isOutput=False)
    bvv = nc.declare_dram_parameter("bvv", [1, E], F16, isOutput=False)
    blv = nc.declare_dram_parameter("blv", [1, E], F16, isOutput=False)
    bvo = nc.declare_dram_parameter("bvo", [1, VD], F16, isOutput=False)
    blo = nc.declare_dram_parameter("blo", [1, LD], F16, isOutput=False)
    maskb = nc.declare_dram_parameter("maskb", [LN, 1], F32, isOutput=False)
    ind2_d = nc.declare_dram_parameter("ind2", [2, 128], F16, isOutput=False)
    oh_d = nc.declare_dram_parameter("oh", [128, 2], F16, isOutput=False)
    oh2_d = nc.declare_dram_parameter("oh2", [128, 2], F16, isOutput=False)
    xv = nc.declare_dram_parameter("xv", [BPC, VN, VD], F32, isOutput=True)
    xl = nc.declare_dram_parameter("xl", [BPC, LN, LD], F32, isOutput=True)

    with tile.TileContext(nc) as tc:
        with (
            tc.tile_pool(name="wp", bufs=1) as wp,
            tc.tile_pool(name="cp", bufs=1) as cp,
            tc.tile_pool(name="ap", bufs=1) as ap_pool,
            tc.tile_pool(name="ep", bufs=4) as ep,
            tc.tile_pool(name="rp", bufs=2) as rp,
            tc.tile_pool(name="op", bufs=2) as op_pool,
            tc.tile_pool(name="ps", bufs=3, space="PSUM") as ps,
            tc.tile_pool(name="pss", bufs=1, space="PSUM") as pss,
        ):
            # ---- resident weights ----
            wvq_sb = [wp.tile([128, E], F16, tag=f"wvq{k}", name=f"wvq{k}") for k in range(8)]
            wvv_sb = [wp.tile([128, E], F16, tag=f"wvv{k}", name=f"wvv{k}") for k in range(8)]
            wlk_sb = [wp.tile([128, E], F16, tag=f"wlk{k}", name=f"wlk{k}") for k in range(6)]
            wlv_sb = [wp.tile([128, E], F16, tag=f"wlv{k}", name=f"wlv{k}") for k in range(6)]
            wvo_sb = [wp.tile([128, VD], F16, tag=f"wvo{k}", name=f"wvo{k}") for k in range(8)]
            wlo_sb = [wp.tile([128, LD], F16, tag=f"wlo{k}", name=f"wlo{k}") for k in range(8)]
            lt_sb = [wp.tile([128, BPC * LN], F16, tag=f"lt{k}", name=f"lt{k}") for k in range(6)]
            for k in range(8):
                nc.sync.dma_start(wvq_sb[k][:], wvq[k * 128:(k + 1) * 128, :])
                nc.sync.dma_start(wvv_sb[k][:], wvv[k * 128:(k + 1) * 128, :])
                nc.sync.dma_start(wvo_sb[k][:], wvo[k * 128:(k + 1) * 128, :])
                nc.sync.dma_start(wlo_sb[k][:], wlo[k * 128:(k + 1) * 128, :])
            for k in range(6):
                nc.sync.dma_start(wlk_sb[k][:], wlk[k * 128:(k + 1) * 128, :])
                nc.sync.dma_start(wlv_sb[k][:], wlv[k * 128:(k + 1) * 128, :])
                nc.sync.dma_start(lt_sb[k][:], lT[k * 128:(k + 1) * 128, :])
            bq = cp.tile([1, E], F16, tag="bq", name="bq")
            bk = cp.tile([1, E], F16, tag="bk", name="bk")
            bv = cp.tile([1, E], F16, tag="bv", name="bv")
            bl = cp.tile([1, E], F16, tag="bl", name="bl")
            bo_v = cp.tile([1, VD], F16, tag="bo_v", name="bo_v")
            bo_l = cp.tile([1, LD], F16, tag="bo_l", name="bo_l")
            nc.sync.dma_start(bq[:], bvq[:])
            nc.sync.dma_start(bk[:], blk[:])
            nc.sync.dma_start(bv[:], bvv[:])
            nc.sync.dma_start(bl[:], blv[:])
            nc.sync.dma_start(bo_v[:], bvo[:])
            nc.sync.dma_start(bo_l[:], blo[:])
            mb = cp.tile([LN, 1], F32, tag="mb", name="mb")
            nc.sync.dma_start(mb[:], maskb[:])
            m4 = cp.tile([128, 1], F32, tag="m4", name="m4")
            nc.vector.memset(m4[:], -SHIFT)

            # ---- constants ----
            ones = cp.tile([1, E], F16, tag="ones", name="ones")
            nc.vector.memset(ones[:], 1.0)
            ind2 = cp.tile([2, 128], F16, tag="ind2", name="ind2")
            oh = cp.tile([128, 2], F16, tag="oh", name="oh")
            oh2 = cp.tile([128, 2], F16, tag="oh2", name="oh2")
            nc.sync.dma_start(ind2[:], ind2_d[:])
            nc.sync.dma_start(oh[:], oh_d[:])
            nc.sync.dma_start(oh2[:], oh2_d[:])

            # ---- kT for all 4 batches:  kT[e, 4*128] ----
            kt = [wp.tile([128, BPC * LN], F16, tag=f"kt{m}", name=f"kt{m}") for m in range(8)]
            for m in range(8):
                acc = ps.tile([128, BPC * LN], F32, tag="ps", name="ps")
                for k in range(6):
                    nc.tensor.matmul(acc[:], wlk_sb[k][:, m * 128:(m + 1) * 128],
                                     lt_sb[k][:], start=(k == 0), stop=False)
                nc.tensor.matmul(acc[:], bk[0:1, m * 128:(m + 1) * 128],
                                 ones[0:1, 0:BPC * LN], start=False, stop=True)
                nc.vector.tensor_copy(kt[m][:], acc[:])

            # ---- lv (natural [s, e]) for all 4 batches ----
            lv = [[wp.tile([128, 512], F16, tag=f"lv{b}_{i}", name=f"lv{b}_{i}") for i in range(2)]
                  for b in range(BPC)]
            for b in range(BPC):
                for i, (n0, nn) in enumerate(N1024):
                    acc = ps.tile([128, 512], F32, tag="ps", name="ps")
                    for k in range(6):
                        nc.tensor.matmul(acc[0:128, 0:nn],
                                         lt_sb[k][:, b * 128:(b + 1) * 128],
                                         wlv_sb[k][:, n0:n0 + nn],
                                         start=(k == 0), stop=False)
                    nc.tensor.matmul(acc[0:128, 0:nn], ones[0:1, 0:128],
                                     bl[0:1, n0:n0 + nn], start=False, stop=True)
                    nc.vector.tensor_copy(lv[b][i][:], acc[0:128, 0:nn])

            # ---- per-batch ----
            for b in range(DBG_BPC or BPC):
                # load vT_b
                vt = [ap_pool.tile([128, VN], F16, tag=f"vt{k}", name=f"vt{k}") for k in range(8)]
                for k in range(8):
                    nc.sync.dma_start(vt[k][:], vT[b, k * 128:(k + 1) * 128, :])

                # qT_b [e, t] (scale folded into wvq/bvq on host)
                qt = [ap_pool.tile([128, VN], F16, tag=f"qt{m}", name=f"qt{m}") for m in range(8)]
                for m in range(8):
                    acc = ps.tile([128, VN], F32, tag="ps", name="ps")
                    for n0, nn in N576:
                        for k in range(8):
                            nc.tensor.matmul(acc[:, n0:n0 + nn],
                                             wvq_sb[k][:, m * 128:(m + 1) * 128],
                                             vt[k][:, n0:n0 + nn],
                                             start=(k == 0), stop=False)
                        nc.tensor.matmul(acc[:, n0:n0 + nn],
                                         bq[0:1, m * 128:(m + 1) * 128],
                                         ones[0:1, 0:nn], start=False, stop=True)
                    nc.vector.tensor_copy(qt[m][:], acc[:])

                # vv_b natural [t, e]
                vv = [ap_pool.tile([128, E], F16, tag=f"vv{c}", name=f"vv{c}") for c in range(5)]
                for c, (c0, cn) in enumerate(TC):
                    acc = ps.tile([128, E], F32, tag="ps", name="ps")
                    for n0, nn in N1024:
                        for k in range(8):
                            nc.tensor.matmul(acc[0:cn, n0:n0 + nn],
                                             vt[k][:, c0:c0 + cn],
                                             wvv_sb[k][:, n0:n0 + nn],
                                             start=(k == 0), stop=False)
                        nc.tensor.matmul(acc[0:cn, n0:n0 + nn], ones[0:1, 0:cn],
                                         bv[0:1, n0:n0 + nn], start=False, stop=True)
                    nc.vector.tensor_copy(vv[c][:cn, :], acc[0:cn, :])

                # attention, head pairs; per-pair sums, ACT log/exp reciprocal
                xvn = [ap_pool.tile([128, VN], F16, tag=f"xvn{j}", name=f"xvn{j}") for j in range(8)]
                xln = [ap_pool.tile([128, LN], F16, tag=f"xln{j}", name=f"xln{j}") for j in range(8)]
                for j in range(DBG_PAIRS or 8):
                    xv_u = ps.tile([128, 640], F32, tag="ps", name="xv_u")
                    xl_u = ps.tile([128, LN], F32, tag="ps", name="xl_u")
                    svl = pss.tile([2, 832], F32, tag="sum", name="svl")
                    for p in range(2):
                        h = 2 * j + p
                        hb = p * 64
                        ohp = oh if p == 0 else oh2
                        kth = kt[j][hb:hb + 64, b * 128:(b + 1) * 128]
                        qth = qt[j][hb:hb + 64, :]

                        # S_T [s, t] then E_v = exp(S_T + mask - SHIFT), fp16
                        st = ps.tile([128, 640], F32, tag="ps", name="st")
                        for n0, nn in N576:
                            nc.tensor.matmul(st[:, n0:n0 + nn], kth, qth[:, n0:n0 + nn],
                                             start=True, stop=True)
                        nc.vector.memset(st[:, VN:640], 0.0)
                        ev = ep.tile([128, 640], F16, tag="ev", name="ev")
                        nc.scalar.activation(ev[:], st[:], EXP, bias=mb[:], scale=1.0)

                        # S2 [t, s] then E2 = exp(S2 - SHIFT)  (no mask: attn_l)
                        s2 = ps.tile([128, 5 * 128], F32, tag="ps", name="s2")
                        for c, (c0, cn) in enumerate(TC):
                            nc.tensor.matmul(s2[0:cn, c * 128:(c + 1) * 128],
                                             qth[:, c0:c0 + cn], kth,
                                             start=True, stop=True)
                        e2 = ep.tile([128, 5 * 128], F16, tag="e2", name="e2")
                        nc.scalar.activation(e2[:], s2[:], EXP, bias=m4[:], scale=1.0)

                        # sumexp rows (bank-window discipline: one clearer per bank)
                        nc.tensor.matmul(svl[0:2, 0:512], ohp, ev[:, 0:512],
                                         start=(p == 0), stop=(p == 1),
                                         skip_group_check=True)
                        mm_sv2 = nc.tensor.matmul(svl[0:2, 576:704], ohp, ev[:, 512:640],
                                                  start=(p == 0), stop=False,
                                                  skip_group_check=True)
                        if p == 0:
                            sv2_first = mm_sv2
                        for c, (c0, cn) in enumerate(TC):
                            mm_sl = nc.tensor.matmul(svl[0:2, 704:704 + LN], ohp[0:cn, :],
                                                     e2[0:cn, c * 128:(c + 1) * 128],
                                                     start=False,
                                                     stop=(p == 1 and c == 4),
                                                     skip_group_check=True)
                            add_dep_helper(mm_sl.ins, sv2_first.ins, sync=False,
                                           reason="bank7 window: clear-first")

                        # PV_v: xv_u[hb:hb+64, t] = lv_h^T(as lhsT) @ E_v
                        lvh = lv[b][h // 8][:, (h % 8) * 64:(h % 8) * 64 + 64]
                        for n0, nn in [(0, 512), (512, 128)]:
                            nc.tensor.matmul(xv_u[hb:hb + 64, n0:n0 + nn], lvh,
                                             ev[:, n0:n0 + nn], start=True, stop=True)
                        # PV_l: xl_u[hb:hb+64, s] = sum_c vv_h_c^T @ E2_c
                        for c, (c0, cn) in enumerate(TC):
                            mm_xl = nc.tensor.matmul(xl_u[hb:hb + 64, :],
                                                     vv[c][0:cn, h * 64:h * 64 + 64],
                                                     e2[0:cn, c * 128:(c + 1) * 128],
                                                     start=(p == 0 and c == 0),
                                                     stop=(p == 1 and c == 4),
                                                     skip_group_check=True)
                            if p == 0 and c == 0:
                                xl_first = mm_xl
                            else:
                                add_dep_helper(mm_xl.ins, xl_first.ins, sync=False,
                                               reason="xl_u bank window: clear-first")

                    # reciprocal of sums on ACT: rcp = exp(-log(sums)), f16 out
                    lgt = rp.tile([2, 832], F32, tag="lgt", name="lgt")
                    nc.scalar.activation(lgt[:], svl[0:2, 0:832], LOG, bias=0.0, scale=1.0)
                    rcp = rp.tile([2, 832], F16, tag="rcp", name="rcp")
                    nc.scalar.activation(rcp[:], lgt[:], EXP, bias=0.0, scale=-1.0)

                    bcv = ps.tile([128, 640], F32, tag="ps", name="bcv")
                    nc.tensor.matmul(bcv[:, 0:512], ind2[:], rcp[:, 0:512],
                                     start=True, stop=True)
                    nc.tensor.matmul(bcv[:, 512:576], ind2[:], rcp[:, 576:640],
                                     start=True, stop=True)
                    bcv_sb = rp.tile([128, VN], F16, tag="bcv_sb", name="bcv_sb")
                    nc.vector.tensor_copy(bcv_sb[:], bcv[:, 0:VN])
                    nc.vector.tensor_tensor(xvn[j][:], xv_u[:, 0:VN], bcv_sb[:],
                                            op=mybir.AluOpType.mult)
                    bcl = ps.tile([128, LN], F32, tag="ps", name="bcl")
                    nc.tensor.matmul(bcl[:], ind2[:], rcp[:, 704:832], start=True, stop=True)
                    bcl_sb = rp.tile([128, LN], F16, tag="bcl_sb", name="bcl_sb")
                    nc.vector.tensor_copy(bcl_sb[:], bcl[:])
                    nc.vector.tensor_tensor(xln[j][:], xl_u[:], bcl_sb[:],
                                            op=mybir.AluOpType.mult)

                # xv out-projection -> natural [t, vd]
                if DBG_PAIRS:
                    continue
                for c, (c0, cn) in enumerate(TC):
                    oev = op_pool.tile([128, VD], F32, tag="oev", name="oev")
                    for n0, nn in N1024:
                        acc = ps.tile([128, 512], F32, tag="ps", name="ps")
                        for jj in range(8):
                            nc.tensor.matmul(acc[0:cn, 0:nn],
                                             xvn[jj][:, c0:c0 + cn],
                                             wvo_sb[jj][:, n0:n0 + nn],
                                             start=(jj == 0), stop=False)
                        nc.tensor.matmul(acc[0:cn, 0:nn], ones[0:1, 0:cn],
                                         bo_v[0:1, n0:n0 + nn], start=False, stop=True)
                        nc.scalar.copy(oev[0:cn, n0:n0 + nn], acc[0:cn, 0:nn])
                    nc.sync.dma_start(xv[b, c0:c0 + cn, :], oev[0:cn, :])

                # xl out-projection -> natural [s, ld]
                oel = op_pool.tile([128, LD], F32, tag="oel", name="oel")
                for n0, nn in N768:
                    acc = ps.tile([128, 512], F32, tag="ps", name="ps")
                    for jj in range(8):
                        nc.tensor.matmul(acc[:, 0:nn], xln[jj][:],
                                         wlo_sb[jj][:, n0:n0 + nn],
                                         start=(jj == 0), stop=False)
                    nc.tensor.matmul(acc[:, 0:nn], ones[0:1, 0:128],
                                     bo_l[0:1, n0:n0 + nn], start=False, stop=True)
                    nc.scalar.copy(oel[:, n0:n0 + nn], acc[:, 0:nn])
                nc.sync.dma_start(xl[b][:], oel[:])

    nc.compile()
    return nc


def kernel(v, l, attention_mask_l, Wvq, bvq, Wlk, blk, Wvv, bvv, Wlv, blv,
           Wvo, bvo, Wlo, blo):
    global _cached_nc
    from concourse.bass_utils import run_bass_kernel_spmd

    if _cached_nc is None:
        _cached_nc = _build_nc()
    nc = _cached_nc

    f16 = np.float16
    v = np.asarray(v, dtype=np.float32)
    l = np.asarray(l, dtype=np.float32)
    shared = {
        "wvq": np.ascontiguousarray((SCALE * np.asarray(Wvq, np.float32).T).astype(f16)),
        "wlk": np.ascontiguousarray(np.asarray(Wlk, np.float32).T.astype(f16)),
        "wvv": np.ascontiguousarray(np.asarray(Wvv, np.float32).T.astype(f16)),
        "wlv": np.ascontiguousarray(np.asarray(Wlv, np.float32).T.astype(f16)),
        "wvo": np.ascontiguousarray(np.asarray(Wvo, np.float32).T.astype(f16)),
        "wlo": np.ascontiguousarray(np.asarray(Wlo, np.float32).T.astype(f16)),
        "bvq": (SCALE * np.asarray(bvq, np.float32)).reshape(1, E).astype(f16),
        "blk": np.asarray(blk, np.float32).reshape(1, E).astype(f16),
        "bvv": np.asarray(bvv, np.float32).reshape(1, E).astype(f16),
        "blv": np.asarray(blv, np.float32).reshape(1, E).astype(f16),
        "bvo": np.asarray(bvo, np.float32).reshape(1, VD).astype(f16),
        "blo": np.asarray(blo, np.float32).reshape(1, LD).astype(f16),
        "maskb": (np.asarray(attention_mask_l, np.float32) - SHIFT)
                 .reshape(LN, 1).astype(np.float32),
        "ind2": np.kron(np.eye(2), np.ones((1, 64))).astype(f16),
        "oh": np.stack([np.ones(128), np.zeros(128)], axis=1).astype(f16),
        "oh2": np.stack([np.zeros(128), np.ones(128)], axis=1).astype(f16),
    }
    in_maps = []
    for c in range(NCORES):
        vc = v[c * BPC:(c + 1) * BPC]
        lc = l[c * BPC:(c + 1) * BPC]
        m = dict(shared)
        m["vT"] = np.ascontiguousarray(vc.transpose(0, 2, 1)).astype(f16)
        m["lT"] = np.ascontiguousarray(lc.transpose(2, 0, 1).reshape(LD, BPC * LN)).astype(f16)
        in_maps.append(m)

    global _last_in_maps
    _last_in_maps = in_maps
    res = run_bass_kernel_spmd(nc, in_maps, list(range(NCORES)))
    global _last_res
    _last_res = res.results[0]
    xv = np.concatenate([res.results[c]["xv"] for c in range(NCORES)], axis=0)
    xl = np.concatenate([res.results[c]["xl"] for c in range(NCORES)], axis=0)
    return xv, xl


# revision 31
# speedup vs baseline: 1.3475x; 1.1306x over previous
"""BiMultiHeadAttention (vision-language cross attention) on 8 Trainium2 cores.

Strategy: pure data-parallel over batch (32 batches -> 4 per core).
All matmul data in fp16 (11-bit mantissa; errors ~1e-3 on the logit path),
accumulation in fp32 PSUM.  Layout chain keeps the contraction axis on
partitions at every step:

  vT/lT (host-pretransposed, fp16)
    -> qT,kT in [e, t] layout;  vv,lv in natural [t, e] layout
    -> S_T[s,t] and S2[t,s] both by matmul (bidirectional softmax needs both)
    -> E = exp(S - 4) in fp16 (shift keeps exp in fp16 range; softmax-invariant)
    -> PV matmuls produce un-normalized xv^T[e,t], xl^T[e,s] per head pair
    -> per-pair sumexp rows via one-hot matmuls; reciprocal; K=2 broadcast
       matmul builds the per-(head,column) scale tile; DVE multiply normalizes
    -> output projections back to natural layout, bias added as K=1 matmul rows.

The attention mask enters E_v as a per-partition bias (exact for any mask);
attn_l/softmax-over-queries is mask-free per the reference.
"""
import sys
import numpy as np

sys.path.insert(0, "/opt/trn_rl_repo")

B, VN, LN = 32, 576, 128
E, H, D = 1024, 16, 64
VD, LD = 1024, 768
NCORES = 8
BPC = B // NCORES          # batches per core
SCALE = H ** -0.5          # 0.25 (reference scales by num_heads**-0.5)
SHIFT = 4.0                # exp(logit - SHIFT) keeps E in fp16 range
TC = [(0, 128), (128, 128), (256, 128), (384, 128), (512, 64)]   # t chunks of 576
N576 = [(0, 512), (512, 64)]
N1024 = [(0, 512), (512, 512)]
N768 = [(0, 512), (512, 256)]

_cached_nc = None


def _oh16():
    m = np.zeros((128, 256), np.float16)
    for h in range(16):
        m[:, h * 16 + h] = 1.0
    return m


def _ind16():
    m = np.zeros((8, 16, 128), np.float16)
    for j in range(8):
        m[j, 2 * j, 0:64] = 1.0
        m[j, 2 * j + 1, 64:128] = 1.0
    return m
import os as _os
DBG_BPC = int(_os.environ.get("DBG_BPC", "0")) or None   # limit batches built
DBG_PAIRS = int(_os.environ.get("DBG_PAIRS", "0")) or None


def _build_nc():
    import concourse.bacc as bacc
    import concourse.hw_specs as hw_specs
    # Prefer the combined ln+exp ACT table set so alternating Ln/Exp
    # activations don't thrash ACT_TABLE_LOAD (~1.3us per reload).
    if not getattr(hw_specs, "_combined_ln_exp_first", False):
        _orig_get_tables = hw_specs.get_activation_tables

        def _reordered(arch):
            t = dict(_orig_get_tables(arch))   # order == act_info.json index order
            key = "natural_log_exp_and_others"
            if key in t:
                shared = t[key]
                strip = {f for f in shared}
                out = {}
                for k, v in t.items():
                    if k == key:
                        out[k] = v
                    else:
                        out[k] = v - strip if isinstance(v, set) else v
                return out
            return t

        hw_specs.get_activation_tables = _reordered
        bacc.get_activation_tables = _reordered
        hw_specs._combined_ln_exp_first = True
    import concourse.mybir as mybir
    import concourse.tile as tile
    from concourse.tile import add_dep_helper

    F16 = mybir.dt.float16
    F32 = mybir.dt.float32
    EXP = mybir.ActivationFunctionType.Exp
    LOG = mybir.ActivationFunctionType.Ln

    nc = bacc.Bacc("TRN2", target_bir_lowering=False, debug=False)
    vT = nc.declare_dram_parameter("vT", [BPC, VD, VN], F16, isOutput=False)
    lT = nc.declare_dram_parameter("lT", [LD, BPC * LN], F16, isOutput=False)
    wvq = nc.declare_dram_parameter("wvq", [VD, E], F16, isOutput=False)
    wlk = nc.declare_dram_parameter("wlk", [LD, E], F16, isOutput=False)
    wvv = nc.declare_dram_parameter("wvv", [VD, E], F16, isOutput=False)
    wlv = nc.declare_dram_parameter("wlv", [LD, E], F16, isOutput=False)
    wvo = nc.declare_dram_parameter("wvo", [E, VD], F16, isOutput=False)
    wlo = nc.declare_dram_parameter("wlo", [E, LD], F16, isOutput=False)
    bvq = nc.declare_dram_parameter("bvq", [1, E], F16, isOutput=False)
    blk = nc.declare_dram_parameter("blk", [1, E], F16, isOutput=False)
    bvv = nc.declare_dram_parameter("bvv", [1, E], F16, isOutput=False)
    blv = nc.declare_dram_parameter("blv", [1, E], F16, isOutput=False)
    bvo = nc.declare_dram_parameter("bvo", [1, VD], F16, isOutput=False)
    blo = nc.declare_dram_parameter("blo", [1, LD], F16, isOutput=False)
    maskb = nc.declare_dram_parameter("maskb", [LN, 1], F32, isOutput=False)
    ind2_d = nc.declare_dram_parameter("ind2", [2, 128], F16, isOutput=False)
    oh_d = nc.declare_dram_parameter("oh", [128, 2], F16, isOutput=False)
    oh2_d = nc.declare_dram_parameter("oh2", [128, 2], F16, isOutput=False)
    xv = nc.declare_dram_parameter("xv", [BPC, VN, VD], F32, isOutput=True)
    xl = nc.declare_dram_parameter("xl", [BPC, LN, LD], F32, isOutput=True)

    with tile.TileContext(nc) as tc:
        with (
            tc.tile_pool(name="wp", bufs=1) as wp,
            tc.tile_pool(name="cp", bufs=1) as cp,
            tc.tile_pool(name="ap", bufs=1) as ap_pool,
            tc.tile_pool(name="ep", bufs=4) as ep,
            tc.tile_pool(name="rp", bufs=2) as rp,
            tc.tile_pool(name="op", bufs=2) as op_pool,
            tc.tile_pool(name="ps", bufs=3, space="PSUM") as ps,
            tc.tile_pool(name="pss", bufs=1, space="PSUM") as pss,
        ):
            # ---- resident weights ----
            wvq_sb = [wp.tile([128, E], F16, tag=f"wvq{k}", name=f"wvq{k}") for k in range(8)]
            wvv_sb = [wp.tile([128, E], F16, tag=f"wvv{k}", name=f"wvv{k}") for k in range(8)]
            wlk_sb = [wp.tile([128, E], F16, tag=f"wlk{k}", name=f"wlk{k}") for k in range(6)]
            wlv_sb = [wp.tile([128, E], F16, tag=f"wlv{k}", name=f"wlv{k}") for k in range(6)]
            wvo_sb = [wp.tile([128, VD], F16, tag=f"wvo{k}", name=f"wvo{k}") for k in range(8)]
            wlo_sb = [wp.tile([128, LD], F16, tag=f"wlo{k}", name=f"wlo{k}") for k in range(8)]
            lt_sb = [wp.tile([128, BPC * LN], F16, tag=f"lt{k}", name=f"lt{k}") for k in range(6)]
            for k in range(8):
                nc.sync.dma_start(wvq_sb[k][:], wvq[k * 128:(k + 1) * 128, :])
                nc.sync.dma_start(wvv_sb[k][:], wvv[k * 128:(k + 1) * 128, :])
                nc.sync.dma_start(wvo_sb[k][:], wvo[k * 128:(k + 1) * 128, :])
                nc.sync.dma_start(wlo_sb[k][:], wlo[k * 128:(k + 1) * 128, :])
            for k in range(6):
                nc.sync.dma_start(wlk_sb[k][:], wlk[k * 128:(k + 1) * 128, :])
                nc.sync.dma_start(wlv_sb[k][:], wlv[k * 128:(k + 1) * 128, :])
                nc.sync.dma_start(lt_sb[k][:], lT[k * 128:(k + 1) * 128, :])
            bq = cp.tile([1, E], F16, tag="bq", name="bq")
            bk = cp.tile([1, E], F16, tag="bk", name="bk")
            bv = cp.tile([1, E], F16, tag="bv", name="bv")
            bl = cp.tile([1, E], F16, tag="bl", name="bl")
            bo_v = cp.tile([1, VD], F16, tag="bo_v", name="bo_v")
            bo_l = cp.tile([1, LD], F16, tag="bo_l", name="bo_l")
            nc.sync.dma_start(bq[:], bvq[:])
            nc.sync.dma_start(bk[:], blk[:])
            nc.sync.dma_start(bv[:], bvv[:])
            nc.sync.dma_start(bl[:], blv[:])
            nc.sync.dma_start(bo_v[:], bvo[:])
            nc.sync.dma_start(bo_l[:], blo[:])
            mb = cp.tile([LN, 1], F32, tag="mb", name="mb")
            nc.sync.dma_start(mb[:], maskb[:])
            m4 = cp.tile([128, 1], F32, tag="m4", name="m4")
            nc.vector.memset(m4[:], -SHIFT)

            # ---- constants ----
            ones = cp.tile([1, E], F16, tag="ones", name="ones")
            nc.vector.memset(ones[:], 1.0)
            ind2 = cp.tile([2, 128], F16, tag="ind2", name="ind2")
            oh = cp.tile([128, 2], F16, tag="oh", name="oh")
            oh2 = cp.tile([128, 2], F16, tag="oh2", name="oh2")
            nc.sync.dma_start(ind2[:], ind2_d[:])
            nc.sync.dma_start(oh[:], oh_d[:])
            nc.sync.dma_start(oh2[:], oh2_d[:])

            # ---- kT for all 4 batches:  kT[e, 4*128] ----
            kt = [wp.tile([128, BPC * LN], F16, tag=f"kt{m}", name=f"kt{m}") for m in range(8)]
            for m in range(8):
                acc = ps.tile([128, BPC * LN], F32, tag="ps", name="ps")
                for k in range(6):
                    nc.tensor.matmul(acc[:], wlk_sb[k][:, m * 128:(m + 1) * 128],
                                     lt_sb[k][:], start=(k == 0), stop=False)
                nc.tensor.matmul(acc[:], bk[0:1, m * 128:(m + 1) * 128],
                                 ones[0:1, 0:BPC * LN], start=False, stop=True)
                nc.vector.tensor_copy(kt[m][:], acc[:])

            # ---- lv (natural [s, e]) for all 4 batches ----
            lv = [[wp.tile([128, 512], F16, tag=f"lv{b}_{i}", name=f"lv{b}_{i}") for i in range(2)]
                  for b in range(BPC)]
            for b in range(BPC):
                for i, (n0, nn) in enumerate(N1024):
                    acc = ps.tile([128, 512], F32, tag="ps", name="ps")
                    for k in range(6):
                        nc.tensor.matmul(acc[0:128, 0:nn],
                                         lt_sb[k][:, b * 128:(b + 1) * 128],
                                         wlv_sb[k][:, n0:n0 + nn],
                                         start=(k == 0), stop=False)
                    nc.tensor.matmul(acc[0:128, 0:nn], ones[0:1, 0:128],
                                     bl[0:1, n0:n0 + nn], start=False, stop=True)
                    nc.vector.tensor_copy(lv[b][i][:], acc[0:128, 0:nn])

            # ---- per-batch ----
            for b in range(DBG_BPC or BPC):
                # load vT_b
                vt = [ap_pool.tile([128, VN], F16, tag=f"vt{k}", name=f"vt{k}") for k in range(8)]
                for k in range(8):
                    nc.sync.dma_start(vt[k][:], vT[b, k * 128:(k + 1) * 128, :])

                # qT_b [e, t] (scale folded into wvq/bvq on host)
                qt = [ap_pool.tile([128, VN], F16, tag=f"qt{m}", name=f"qt{m}") for m in range(8)]
                for m in range(8):
                    acc = ps.tile([128, VN], F32, tag="ps", name="ps")
                    for n0, nn in N576:
                        for k in range(8):
                            nc.tensor.matmul(acc[:, n0:n0 + nn],
                                             wvq_sb[k][:, m * 128:(m + 1) * 128],
                                             vt[k][:, n0:n0 + nn],
                                             start=(k == 0), stop=False)
                        nc.tensor.matmul(acc[:, n0:n0 + nn],
                                         bq[0:1, m * 128:(m + 1) * 128],
                                         ones[0:1, 0:nn], start=False, stop=True)
                    nc.vector.tensor_copy(qt[m][:], acc[:])

                # vv_b natural [t, e]
                vv = [ap_pool.tile([128, E], F16, tag=f"vv{c}", name=f"vv{c}") for c in range(5)]
                for c, (c0, cn) in enumerate(TC):
                    acc = ps.tile([128, E], F32, tag="ps", name="ps")
                    for n0, nn in N1024:
                        for k in range(8):
                            nc.tensor.matmul(acc[0:cn, n0:n0 + nn],
                                             vt[k][:, c0:c0 + cn],
                                             wvv_sb[k][:, n0:n0 + nn],
                                             start=(k == 0), stop=False)
                        nc.tensor.matmul(acc[0:cn, n0:n0 + nn], ones[0:1, 0:cn],
                                         bv[0:1, n0:n0 + nn], start=False, stop=True)
                    nc.vector.tensor_copy(vv[c][:cn, :], acc[0:cn, :])

                # attention, head pairs; per-pair sums, ACT log/exp reciprocal
                xvn = [ap_pool.tile([128, VN], F16, tag=f"xvn{j}", name=f"xvn{j}") for j in range(8)]
                xln = [ap_pool.tile([128, LN], F16, tag=f"xln{j}", name=f"xln{j}") for j in range(8)]
                for j in range(DBG_PAIRS or 8):
                    xv_u = ps.tile([128, 768], F32, tag="ps", name="xv_u")
                    xl_u = xv_u[:, 640:768]
                    svl = pss.tile([2, 832], F32, tag="sum", name="svl")
                    for p in range(2):
                        h = 2 * j + p
                        hb = p * 64
                        ohp = oh if p == 0 else oh2
                        kth = kt[j][hb:hb + 64, b * 128:(b + 1) * 128]
                        qth = qt[j][hb:hb + 64, :]

                        # S_T [s, t] then E_v = exp(S_T + mask - SHIFT), fp16
                        st = ps.tile([128, 640], F32, tag="ps", name="st")
                        for n0, nn in N576:
                            nc.tensor.matmul(st[:, n0:n0 + nn], kth, qth[:, n0:n0 + nn],
                                             start=True, stop=True)
                        nc.vector.memset(st[:, VN:640], 0.0)
                        ev = ep.tile([128, 640], F16, tag="ev", name="ev")
                        nc.scalar.activation(ev[:], st[:], EXP, bias=mb[:], scale=1.0)

                        # S2 [t, s] then E2 = exp(S2 - SHIFT)  (no mask: attn_l)
                        s2 = ps.tile([128, 5 * 128], F32, tag="ps", name="s2")
                        for c, (c0, cn) in enumerate(TC):
                            nc.tensor.matmul(s2[0:cn, c * 128:(c + 1) * 128],
                                             qth[:, c0:c0 + cn], kth,
                                             start=True, stop=True)
                        e2 = ep.tile([128, 5 * 128], F16, tag="e2", name="e2")
                        nc.scalar.activation(e2[:], s2[:], EXP, bias=m4[:], scale=1.0)

                        # sumexp rows (bank-window discipline: one clearer per bank)
                        nc.tensor.matmul(svl[0:2, 0:512], ohp, ev[:, 0:512],
                                         start=(p == 0), stop=(p == 1),
                                         skip_group_check=True)
                        mm_sv2 = nc.tensor.matmul(svl[0:2, 576:704], ohp, ev[:, 512:640],
                                                  start=(p == 0), stop=False,
                                                  skip_group_check=True)
                        if p == 0:
                            sv2_first = mm_sv2
                        for c, (c0, cn) in enumerate(TC):
                            mm_sl = nc.tensor.matmul(svl[0:2, 704:704 + LN], ohp[0:cn, :],
                                                     e2[0:cn, c * 128:(c + 1) * 128],
                                                     start=False,
                                                     stop=(p == 1 and c == 4),
                                                     skip_group_check=True)
                            add_dep_helper(mm_sl.ins, sv2_first.ins, sync=False,
                                           reason="bank7 window: clear-first")

                        # PV_v: xv_u[hb:hb+64, t] = lv_h^T(as lhsT) @ E_v
                        # PV_l first: (p,c0) clears this parity's rows across bank B
                        for c, (c0, cn) in enumerate(TC):
                            mm_xl = nc.tensor.matmul(xl_u[hb:hb + 64, :],
                                                     vv[c][0:cn, h * 64:h * 64 + 64],
                                                     e2[0:cn, c * 128:(c + 1) * 128],
                                                     start=(c == 0), stop=(c == 4),
                                                     skip_group_check=True)
                            if c == 0:
                                xl_cl = mm_xl
                            else:
                                add_dep_helper(mm_xl.ins, xl_cl.ins, sync=False,
                                               reason="xl rows window: clear-first")
                        # PV_v: slice1 bank A self-clearing; slice2 shares bank B
                        # with xl rows window -> start=False, after this p's clearer
                        lvh = lv[b][h // 8][:, (h % 8) * 64:(h % 8) * 64 + 64]
                        nc.tensor.matmul(xv_u[hb:hb + 64, 0:512], lvh,
                                         ev[:, 0:512], start=True, stop=True)
                        mm_pv2 = nc.tensor.matmul(xv_u[hb:hb + 64, 512:640], lvh,
                                                  ev[:, 512:640], start=False, stop=True,
                                                  skip_group_check=True)
                        add_dep_helper(mm_pv2.ins, xl_cl.ins, sync=False,
                                       reason="xv_u bankB after xl row-clear")

                    # reciprocal of sums on ACT: rcp = exp(-log(sums)), f16 out
                    lgt = rp.tile([2, 832], F32, tag="lgt", name="lgt")
                    nc.scalar.activation(lgt[:], svl[0:2, 0:832], LOG, bias=0.0, scale=1.0)
                    rcp = rp.tile([2, 832], F16, tag="rcp", name="rcp")
                    nc.scalar.activation(rcp[:], lgt[:], EXP, bias=0.0, scale=-1.0)

                    bcv = ps.tile([128, 640], F32, tag="ps", name="bcv")
                    nc.tensor.matmul(bcv[:, 0:512], ind2[:], rcp[:, 0:512],
                                     start=True, stop=True)
                    nc.tensor.matmul(bcv[:, 512:576], ind2[:], rcp[:, 576:640],
                                     start=True, stop=True)
                    bcv_sb = rp.tile([128, VN], F16, tag="bcv_sb", name="bcv_sb")
                    nc.vector.tensor_copy(bcv_sb[:], bcv[:, 0:VN])
                    nc.vector.tensor_tensor(xvn[j][:], xv_u[:, 0:VN], bcv_sb[:],
                                            op=mybir.AluOpType.mult)
                    bcl = ps.tile([128, LN], F32, tag="ps", name="bcl")
                    nc.tensor.matmul(bcl[:], ind2[:], rcp[:, 704:832], start=True, stop=True)
                    bcl_sb = rp.tile([128, LN], F16, tag="bcl_sb", name="bcl_sb")
                    nc.vector.tensor_copy(bcl_sb[:], bcl[:])
                    nc.vector.tensor_tensor(xln[j][:], xl_u[:], bcl_sb[:],
                                            op=mybir.AluOpType.mult)

                # xv out-projection -> natural [t, vd]
                if DBG_PAIRS:
                    continue
                for c, (c0, cn) in enumerate(TC):
                    oev = op_pool.tile([128, VD], F32, tag="oev", name="oev")
                    for n0, nn in N1024:
                        acc = ps.tile([128, 512], F32, tag="ps", name="ps")
                        for jj in range(8):
                            nc.tensor.matmul(acc[0:cn, 0:nn],
                                             xvn[jj][:, c0:c0 + cn],
                                             wvo_sb[jj][:, n0:n0 + nn],
                                             start=(jj == 0), stop=False)
                        nc.tensor.matmul(acc[0:cn, 0:nn], ones[0:1, 0:cn],
                                         bo_v[0:1, n0:n0 + nn], start=False, stop=True)
                        nc.scalar.copy(oev[0:cn, n0:n0 + nn], acc[0:cn, 0:nn])
                    nc.sync.dma_start(xv[b, c0:c0 + cn, :], oev[0:cn, :])

                # xl out-projection -> natural [s, ld]
                oel = op_pool.tile([128, LD], F32, tag="oel", name="oel")
                for n0, nn in N768:
                    acc = ps.tile([128, 512], F32, tag="ps", name="ps")
                    for jj in range(8):
                        nc.tensor.matmul(acc[:, 0:nn], xln[jj][:],
                                         wlo_sb[jj][:, n0:n0 + nn],
                                         start=(jj == 0), stop=False)
                    nc.tensor.matmul(acc[:, 0:nn], ones[0:1, 0:128],
                                     bo_l[0:1, n0:n0 + nn], start=False, stop=True)
                    nc.scalar.copy(oel[:, n0:n0 + nn], acc[:, 0:nn])
                nc.sync.dma_start(xl[b][:], oel[:])

    nc.compile()
    return nc


def kernel(v, l, attention_mask_l, Wvq, bvq, Wlk, blk, Wvv, bvv, Wlv, blv,
           Wvo, bvo, Wlo, blo):
    global _cached_nc
    from concourse.bass_utils import run_bass_kernel_spmd

    if _cached_nc is None:
        _cached_nc = _build_nc()
    nc = _cached_nc

    f16 = np.float16
    v = np.asarray(v, dtype=np.float32)
    l = np.asarray(l, dtype=np.float32)
    shared = {
        "wvq": np.ascontiguousarray((SCALE * np.asarray(Wvq, np.float32).T).astype(f16)),
        "wlk": np.ascontiguousarray(np.asarray(Wlk, np.float32).T.astype(f16)),
        "wvv": np.ascontiguousarray(np.asarray(Wvv, np.float32).T.astype(f16)),
        "wlv": np.ascontiguousarray(np.asarray(Wlv, np.float32).T.astype(f16)),
        "wvo": np.ascontiguousarray(np.asarray(Wvo, np.float32).T.astype(f16)),
        "wlo": np.ascontiguousarray(np.asarray(Wlo, np.float32).T.astype(f16)),
        "bvq": (SCALE * np.asarray(bvq, np.float32)).reshape(1, E).astype(f16),
        "blk": np.asarray(blk, np.float32).reshape(1, E).astype(f16),
        "bvv": np.asarray(bvv, np.float32).reshape(1, E).astype(f16),
        "blv": np.asarray(blv, np.float32).reshape(1, E).astype(f16),
        "bvo": np.asarray(bvo, np.float32).reshape(1, VD).astype(f16),
        "blo": np.asarray(blo, np.float32).reshape(1, LD).astype(f16),
        "maskb": (np.asarray(attention_mask_l, np.float32) - SHIFT)
                 .reshape(LN, 1).astype(np.float32),
        "ind2": np.kron(np.eye(2), np.ones((1, 64))).astype(f16),
        "oh": np.stack([np.ones(128), np.zeros(128)], axis=1).astype(f16),
        "oh2": np.stack([np.zeros(128), np.ones(128)], axis=1).astype(f16),
    }
    in_maps = []
    for c in range(NCORES):
        vc = v[c * BPC:(c + 1) * BPC]
        lc = l[c * BPC:(c + 1) * BPC]
        m = dict(shared)
        m["vT"] = np.ascontiguousarray(vc.transpose(0, 2, 1)).astype(f16)
        m["lT"] = np.ascontiguousarray(lc.transpose(2, 0, 1).reshape(LD, BPC * LN)).astype(f16)
        in_maps.append(m)

    global _last_in_maps
    _last_in_maps = in_maps
    res = run_bass_kernel_spmd(nc, in_maps, list(range(NCORES)))
    global _last_res
    _last_res = res.results[0]
    xv = np.concatenate([res.results[c]["xv"] for c in range(NCORES)], axis=0)
    xl = np.concatenate([res.results[c]["xl"] for c in range(NCORES)], axis=0)
    return xv, xl
